# revision 20
# baseline (speedup 1.0000x reference)
"""Trainium2 Bass kernel for the seq2seq decoder (Bahdanau attention + 2-layer
LSTM + vocab projection).

Strategy (8 NeuronCores):
  - Embedding gather, attention, both LSTM recurrences are replicated on every
    core at full batch B=32 (the recurrence streams Wh through the PE every
    step, so its wall time is independent of the batch shard; per-step
    collectives would cost more than they save).
  - The fc [512, 32000] weight is sharded column-wise (vocab) 8 ways: each
    core computes logits for its 4000-vocab slice => zero collectives.
  - Host reassembles: concat logits slices on vocab, takes hT/cT/pesos from
    core 0.

Matmuls run as float32r (full PE rate for moving dim >= 256). Decoder token
order is t-major (tok = t*32 + b) so per-step rows are contiguous.
"""

import os
import sys

for _p in ("/opt/trn_rl_repo",):
    if _p not in sys.path and os.path.isdir(_p):
        sys.path.insert(0, _p)

from contextlib import ExitStack

import numpy as np

import concourse.bass as bass
import concourse.mybir as mybir
import concourse.tile as tile
from concourse import bacc
from concourse.bass_utils import run_bass_kernel_spmd
from concourse.masks import make_identity

B, S, T = 32, 128, 64
VOC, E, U, EU = 32000, 256, 512, 512
NC = 8
VS = VOC // NC  # 4000 vocab per core
VSP = 4096  # padded vocab slice
TOK = B * T  # 2048 decoder tokens
STOK = B * S  # 4096 encoder tokens
G4 = 4 * U  # 2048 gate width
P = 128

f32 = mybir.dt.float32
f32r = mybir.dt.float32r
i32 = mybir.dt.int32
AF = mybir.ActivationFunctionType
AX = mybir.AxisListType

# All matmul operands are staged in SBUF as float32r (walrus requires fp32r
# matmul inputs to be produced/rounded as fp32r, so tiles are typed f32r and
# the producing DMA/ACT op converts on write).


def _r(ap, group):
    return ap


def build_program(bias_zero=True):
    global _BIAS_ZERO
    _BIAS_ZERO = bias_zero
    nc = bacc.Bacc("TRN2", target_bir_lowering=False, debug=False,
                   enable_asserts=False, num_devices=NC)

    d = {}

    def inp(name, shape, dtype=f32):
        d[name] = nc.dram_tensor(name, shape, dtype, kind="ExternalInput").ap()

    def outp(name, shape, dtype=f32):
        d[name] = nc.dram_tensor(name, shape, dtype, kind="ExternalOutput").ap()

    inp("ids", [TOK, 1], i32)
    inp("emb", [VOC, E])
    inp("enc", [STOK, EU], f32r)
    inp("encaugT", [EU + U, STOK], f32r)  # [enc^T ; bcast(h0)^T], tok b-major
    inp("w12", [EU + U, U], f32r)
    inp("va", [P, 8], f32r)  # host-staged [128, (k,pair)] layout
    inp("b12", [U, 1])
    inp("h0", [B, U])
    inp("c0", [B, U])
    inp("wx1b", [E + EU + 1, G4], f32r)
    inp("wh1", [U, G4], f32r)
    inp("wx2b", [U + 1, G4], f32r)
    inp("wh2", [U, G4], f32r)
    inp("wfcb", [U + 1, VSP], f32r)

    outp("logits", [TOK, VSP])
    outp("pesos", [B, S])
    outp("hT", [B, U])
    outp("cT", [B, U])

    with tile.TileContext(nc) as tc:
        with ExitStack() as ctx:
            _build(ctx, tc, nc, d)

    nc.compile()
    return nc


def _build(ctx, tc, nc, d):
    const = ctx.enter_context(tc.tile_pool(name="const", bufs=1))
    ident = const.tile([P, P], f32)
    make_identity(nc, ident)
    ident_r = const.tile([P, P], f32r)
    nc.vector.tensor_copy(ident_r[:], ident[:])
    ones_row = const.tile([1, P], f32)
    nc.any.memset(ones_row[:], 1.0)
    ones_r = const.tile([1, P], f32r)
    nc.vector.tensor_copy(ones_r[:], ones_row[:])
    dummyw = const.tile([P, P], mybir.dt.bfloat16)
    nc.any.memset(dummyw[:], 0.0)

    def pe_keepwarm(n):
        for _ in range(n):
            nc.tensor.ldweights(dummyw[:])

    # small shared PSUM ring (transposes, tiny matmul outputs): 2 banks
    ps_small = ctx.enter_context(tc.tile_pool(name="ps_small", bufs=2, space="PSUM"))

    def psmall():
        return ps_small.tile([P, P], f32, tag="small", name="small")

    # persistent tiles live for the whole kernel
    state = ctx.enter_context(tc.tile_pool(name="state", bufs=1))
    c_sb = state.tile([B, U], f32, tag="c_state")
    nc.sync.dma_start(c_sb[:], d["c0"][:, :])
    h0_sb = state.tile([B, U], f32, tag="h0")
    nc.sync.dma_start(h0_sb[:], d["h0"][:, :])
    h0T = state.tile([P, 4 * B], f32r, tag="h0T")
    # rolling h^T buffers: 8 steps deep (step t -> col block (t % 8))
    h1T = [state.tile([P, 8 * B], f32r, tag=f"h1T{k}", name=f"h1T{k}") for k in range(4)]
    h2T = [state.tile([P, 8 * B], f32r, tag=f"h2T{k}", name=f"h2T{k}") for k in range(4)]
    ctxT = [state.tile([P, B], f32r, tag=f"ctxT{m}", name=f"ctxT{m}") for m in range(4)]

    # h0T: 4 PE transposes of h0 [32,512] -> [128, 4*32]
    for k in range(4):
        tp = psmall()
        nc.tensor.transpose(tp[0:P, 0:B], h0_sb[:, k * P:(k + 1) * P],
                            ident[0:B, 0:B])
        nc.scalar.activation(h0T[:, k * B:(k + 1) * B], tp[0:P, 0:B], AF.Identity)

    bc_pool = tc.tile_pool(name="bc_pool", bufs=1)
    bc = bc_pool.__enter__()
    ctxbc = [bc.tile([P, P], f32r, tag=f"ctxbc{m}", name=f"ctxbc{m}") for m in range(4)]
    xeT = [bc.tile([P, TOK], f32r, tag=f"xeT{c}", name=f"xeT{c}") for c in range(2)]
    with tc.tile_pool(name="embp", bufs=3) as embp:
        for i in range(TOK // P):
            idx = embp.tile([P, 1], i32, tag="idx")
            nc.sync.dma_start(idx[:], d["ids"][i * P:(i + 1) * P, :])
            xe = embp.tile([P, E], f32, tag="xe")
            nc.gpsimd.indirect_dma_start(
                out=xe[:], out_offset=None, in_=d["emb"][:, :],
                in_offset=bass.IndirectOffsetOnAxis(ap=idx[:, 0:1], axis=0))
            for c in range(2):
                tp = psmall()
                nc.tensor.transpose(tp[:], xe[:, c * P:(c + 1) * P], ident[:])
                nc.scalar.activation(xeT[c][:, i * P:(i + 1) * P], tp[:], AF.Identity)


    # ---------------- Phase A: attention ----------------
    with tc.tile_pool(name="attn", bufs=1) as attn, \
         tc.tile_pool(name="attn_mov", bufs=2) as attn_mov, \
         tc.tile_pool(name="up_ps", bufs=2, space="PSUM") as up_ps, \
         tc.tile_pool(name="ctx_ps", bufs=1, space="PSUM") as ctx_ps:

        w12_sb = [attn.tile([P, U], f32r, tag=f"w12_{k}", name=f"w12_{k}") for k in range(8)]
        for k in range(8):
            nc.sync.dma_start(w12_sb[k][:], d["w12"][k * P:(k + 1) * P, :])
        va_sb = attn.tile([P, 8], f32r, tag="va")
        nc.sync.dma_start(va_sb[:], d["va"][:, :])
        b12_sb = attn.tile([P, 4], f32, tag="b12")
        nc.sync.dma_start(b12_sb[:], d["b12"][:, 0:1].rearrange("(k p) o -> p (k o)", p=P))

        tanhuT = [attn.tile([P, STOK], f32r, tag=f"tanhuT{m}", name=f"tanhuT{m}") for m in range(4)]

        # u^T GEMM: [512 u, 4096 tok] = w12^T-chunks @ encaugT, + tanh fused
        for n in range(8):
            mov = [attn_mov.tile([P, 512], f32r, tag=f"amov{k}", name=f"amov{k}") for k in range(8)]
            for k in range(8):
                nc.sync.dma_start(mov[k][:], d["encaugT"][k * P:(k + 1) * P,
                                                          n * 512:(n + 1) * 512])
            for m in range(4):
                ps = up_ps.tile([P, 512], f32, tag="ups")
                for k in range(8):
                    nc.tensor.matmul(ps[:], _r(w12_sb[k][:, m * P:(m + 1) * P], "attn"),
                                     _r(mov[k][:], "attn"),
                                     start=(k == 0), stop=(k == 7))
                nc.scalar.activation(tanhuT[m][:, n * 512:(n + 1) * 512], ps[:],
                                     AF.Tanh, bias=b12_sb[:, m:m + 1])

        # score^T [128 s, 32 b]
        scoreT = attn.tile([P, B], f32, tag="scoreT")
        for b in range(B):
            ps = psmall()
            for k in range(4):
                nc.tensor.matmul(ps[0:P, 0:2],
                                 _r(tanhuT[k][:, b * P:(b + 1) * P], "attn"),
                                 _r(va_sb[:, 2 * k:2 * k + 2], "attn"),
                                 start=(k == 0), stop=(k == 3))
            nc.scalar.activation(scoreT[:, b:b + 1], ps[0:P, 0:1], AF.Identity)

        # softmax over s: transpose to [32 b, 128 s]
        ps2 = psmall()
        nc.tensor.transpose(ps2[0:B, 0:P], scoreT[:], ident[:])
        sc = attn.tile([B, P], f32, tag="sc")
        nc.scalar.activation(sc[:], ps2[0:B, 0:P], AF.Identity)
        mx = attn.tile([B, 1], f32, tag="mx")
        nc.vector.reduce_max(mx[:], sc[:], axis=AX.X)
        negmx = attn.tile([B, 1], f32, tag="negmx")
        nc.vector.tensor_scalar_mul(negmx[:], mx[:], -1.0)
        pes = attn.tile([B, P], f32, tag="pes")
        sm = attn.tile([B, 1], f32, tag="sm")
        nc.scalar.activation(pes[:], sc[:], AF.Exp, bias=negmx[:, 0:1],
                             accum_out=sm[:, 0:1])
        rs = attn.tile([B, 1], f32, tag="rs")
        nc.vector.reciprocal(rs[:], sm[:])
        pesos_sb = attn.tile([B, P], f32, tag="pesos")
        nc.vector.tensor_scalar_mul(pesos_sb[:], pes[:], rs[:, 0:1])
        nc.sync.dma_start(d["pesos"][:, :], pesos_sb[:])

        # pesos^T [128 s, 32 b]
        ps3 = psmall()
        nc.tensor.transpose(ps3[0:P, 0:B], pesos_sb[:], ident[0:B, 0:B])
        pesT = attn.tile([P, B + 2], f32r, tag="pesT")
        nc.scalar.activation(pesT[:, 0:B], ps3[0:P, 0:B], AF.Identity)
        # keep the overhang pair finite (read, never used)
        nc.scalar.activation(pesT[:, B:B + 2], ps3[0:P, 0:2], AF.Identity)

        # ctx^T [512 e, 32 b]: per-b matvecs against native enc[b] tiles
        # psum columns strided by 2 to keep matmul outputs 8-byte aligned
        ctxps = [ctx_ps.tile([P, 2 * B], f32, tag=f"ctxps{m}", name=f"ctxps{m}") for m in range(4)]
        for b in range(B):
            encb = attn_mov.tile([P, EU], f32r, tag="encb")
            nc.sync.dma_start(encb[:], d["enc"][b * P:(b + 1) * P, :])
            for m in range(4):
                nc.tensor.matmul(ctxps[m][:, 2 * b:2 * b + 2],
                                 _r(encb[:, m * P:(m + 1) * P], "attn"),
                                 _r(pesT[:, b:b + 2], "attn"),
                                 start=True, stop=True)
        for m in range(4):
            nc.scalar.activation(ctxT[m][:], ctxps[m][:, 0:2 * B:2], AF.Identity)

    # ctx broadcast tiles [128, 128]: col (t%4)*32+b = ctxT[:, b]
    for m in range(4):
        for j in range(4):
            nc.vector.tensor_copy(ctxbc[m][:, j * B:(j + 1) * B], ctxT[m][:])

    # ---------------- Phase C: z1x = [xe|ctx|1] @ [Wx1; bl1] -> DRAM ------
    dram = ctx.enter_context(tc.tile_pool(name="dram", bufs=1, space="DRAM"))
    z1x_d = dram.tile([TOK, G4], f32r, tag="z1x")
    z2x_d = dram.tile([TOK, G4], f32r, tag="z2x")

    with tc.tile_pool(name="z1xw", bufs=1) as z1xw, \
         tc.tile_pool(name="z1xev", bufs=4) as z1xev, \
         tc.tile_pool(name="z1xps", bufs=4, space="PSUM") as z1xps:
        wx1_sb = [z1xw.tile([P, G4], f32r, tag=f"wx1_{k}", name=f"wx1_{k}") for k in range(6)]
        for k in range(6):
            nc.sync.dma_start(wx1_sb[k][:], d["wx1b"][k * P:(k + 1) * P, :])
        bl1_sb = z1xw.tile([1, G4], f32r, tag="bl1")
        nc.sync.dma_start(bl1_sb[:], d["wx1b"][E + EU:E + EU + 1, :])

        for m in range(TOK // P):
            for g in range(4):
                ps = z1xps.tile([P, 512], f32, tag="zps")
                gs = slice(g * 512, (g + 1) * 512)
                for k in range(2):
                    nc.tensor.matmul(ps[:], _r(xeT[k][:, m * P:(m + 1) * P], "zx"),
                                     _r(wx1_sb[k][:, gs], "zx"),
                                     start=(k == 0), stop=False)
                for k in range(4):
                    nc.tensor.matmul(ps[:], _r(ctxbc[k][:], "zx"),
                                     _r(wx1_sb[2 + k][:, gs], "zx"),
                                     start=False, stop=(k == 3 and _BIAS_ZERO))
                if not _BIAS_ZERO:
                    nc.tensor.matmul(ps[:], _r(ones_r[:, 0:P], "zx"),
                                     _r(bl1_sb[:, gs], "zx"), start=False,
                                     stop=True)
                ev = z1xev.tile([P, 512], f32r, tag="zev")
                nc.vector.tensor_copy(ev[:], ps[:])
                nc.sync.dma_start(z1x_d[m * P:(m + 1) * P, gs], ev[:])

    bc_pool.__exit__(None, None, None)

    # ---------------- Phase D/E: LSTM layers with interleaved GEMMs -------
    with tc.tile_pool(name="lstm_sb", bufs=1) as lstm_sb, \
         tc.tile_pool(name="lstm_zx", bufs=3) as lstm_zx, \
         tc.tile_pool(name="lstm_ps", bufs=1, space="PSUM") as lstm_ps, \
         tc.tile_pool(name="lstm_tp", bufs=2, space="PSUM") as lstm_tp, \
         tc.tile_pool(name="w2pool", bufs=1) as w2pool:

        # prefetch wh2 early (free DMA time during layer 1)
        wh2_sb = [w2pool.tile([P, G4], f32r, tag=f"wh2_{k}", name=f"wh2_{k}")
                  for k in range(4)]
        for k in range(4):
            nc.sync.dma_start(wh2_sb[k][:], d["wh2"][k * P:(k + 1) * P, :])

        def lstm_ids(t, zx_dram):
            """Emit the z-identity matmuls (no h^T dependency)."""
            zx = lstm_zx.tile([B, G4], f32r, tag="zx", name="zx")
            nc.sync.dma_start(zx[:], zx_dram[t * B:(t + 1) * B, :])
            gps = {}
            for g in (1, 0, 2, 3):
                ps = lstm_ps.tile([B, 512], f32, tag=f"gate{g}", name=f"gate{g}")
                gs = slice(g * 512, (g + 1) * 512)
                nc.tensor.matmul(ps[:], ident_r[0:B, 0:B], zx[:, gs],
                                 start=True, stop=False)
                gps[g] = ps
            pe_keepwarm(8)
            return gps

        def lstm_step(t, gps, wh_sb, hT_prev_ap, hT_out):
            """Wh matmuls + cell; h^T transposes emitted separately."""
            for g in (1, 0, 2, 3):
                ps = gps[g]
                gs = slice(g * 512, (g + 1) * 512)
                for k in range(4):
                    nc.tensor.matmul(ps[:], hT_prev_ap(k), wh_sb[k][:, gs],
                                     start=False, stop=(k == 3))
            gi = lstm_sb.tile([B, 512], f32, tag="gi")
            gf = lstm_sb.tile([B, 512], f32, tag="gf")
            gg = lstm_sb.tile([B, 512], f32, tag="gg")
            go = lstm_sb.tile([B, 512], f32, tag="go")
            t1 = lstm_sb.tile([B, 512], f32, tag="t1")
            t2 = lstm_sb.tile([B, 512], f32, tag="t2")
            tcl = lstm_sb.tile([B, 512], f32, tag="tc")
            h = lstm_sb.tile([B, 512], f32, tag="h", bufs=2)
            r = (t % 8) * B
            nc.scalar.activation(gf[:], gps[1][:], AF.Sigmoid)
            nc.scalar.activation(gi[:], gps[0][:], AF.Sigmoid)
            nc.scalar.activation(gg[:], gps[2][:], AF.Tanh)
            nc.scalar.activation(go[:], gps[3][:], AF.Sigmoid)
            del gps
            nc.vector.tensor_mul(t1[:], gf[:], c_sb[:])
            nc.vector.tensor_mul(t2[:], gi[:], gg[:])
            nc.vector.tensor_add(c_sb[:], t1[:], t2[:])
            nc.scalar.activation(tcl[:], c_sb[:], AF.Tanh)
            nc.vector.tensor_mul(h[:], go[:], tcl[:])
            return h

        def lstm_hT(t, h, hT_out):
            r = (t % 8) * B
            for k in range(4):
                tp = lstm_tp.tile([P, B], f32, tag="htp", name="htp")
                nc.tensor.transpose(tp[:], h[:, k * P:(k + 1) * P],
                                    ident[0:B, 0:B])
                nc.scalar.activation(hT_out[k][:, r:r + B], tp[:], AF.Identity)
                pe_keepwarm(2)

        # ---- layer 1, with z2x chunks interleaved every 4 steps ----
        with tc.tile_pool(name="wpool", bufs=1) as wpool:
            wh1_sb = [wpool.tile([P, G4], f32r, tag=f"wh1_{k}", name=f"wh1_{k}")
                      for k in range(4)]
            wx2_sb = [wpool.tile([P, G4], f32r, tag=f"wx2_{k}", name=f"wx2_{k}")
                      for k in range(4)]
            for k in range(4):
                nc.sync.dma_start(wh1_sb[k][:], d["wh1"][k * P:(k + 1) * P, :])
                nc.sync.dma_start(wx2_sb[k][:], d["wx2b"][k * P:(k + 1) * P, :])
            bl2_sb = wpool.tile([1, G4], f32r, tag="bl2")
            nc.sync.dma_start(bl2_sb[:], d["wx2b"][U:U + 1, :])

            def z2x_unit_l1(q):
                # one gate-chunk of z2x token-chunk m (steps 4m..4m+3)
                m, g = q // 4, q % 4
                r0 = ((4 * m) % 8) * B
                ps = ps_small.tile([P, 512], f32, tag="small", name=f"z2ps{g}")
                gs = slice(g * 512, (g + 1) * 512)
                for k in range(4):
                    nc.tensor.matmul(ps[:], h1T[k][:, r0:r0 + P],
                                     wx2_sb[k][:, gs],
                                     start=(k == 0),
                                     stop=(k == 3 and _BIAS_ZERO))
                if not _BIAS_ZERO:
                    nc.tensor.matmul(ps[:], ones_r[:, 0:P], bl2_sb[:, gs],
                                     start=False, stop=True)
                ev = lstm_sb.tile([P, 512], f32r, tag="z2ev", bufs=2, name="z2ev")
                nc.vector.tensor_copy(ev[:], ps[:])
                nc.sync.dma_start(z2x_d[m * P:(m + 1) * P, gs], ev[:])

            h_prev_l1 = None
            for t in range(T):
                if t == 0:
                    prev = lambda k: h0T[:, k * B:(k + 1) * B]
                else:
                    prev = lambda k, _t=t: h1T[k][:, ((_t - 1) % 8) * B:
                                                  (((_t - 1) % 8) + 1) * B]
                if t >= 5:
                    z2x_unit_l1(t - 5)
                gps_t = lstm_ids(t, z1x_d)
                if h_prev_l1 is not None:
                    lstm_hT(t - 1, h_prev_l1, h1T)
                h_t = lstm_step(t, gps_t, wh1_sb, prev, h1T)
                h_prev_l1 = h_t

            lstm_hT(T - 1, h_prev_l1, h1T)
            for q in range(T - 5, T):
                z2x_unit_l1(q)

        # ---- layer 2, with FC chunks interleaved every 4 steps ----
        with tc.tile_pool(name="fcw", bufs=1) as fcw:
            wfc_sb = [fcw.tile([P, VSP], f32r, tag=f"wfc_{k}", name=f"wfc_{k}")
                      for k in range(4)]
            for k in range(4):
                nc.sync.dma_start(wfc_sb[k][:], d["wfcb"][k * P:(k + 1) * P, :])
            bfc_sb = fcw.tile([1, VSP], f32r, tag="bfc")
            nc.sync.dma_start(bfc_sb[:], d["wfcb"][U:U + 1, :])

            def fc_unit2(q):
                # one vocab-chunk of logits token-chunk m (steps 4m..4m+3)
                m, v = q // 8, q % 8
                r0 = ((4 * m) % 8) * B
                ps = ps_small.tile([P, 512], f32, tag="small", name="fps")
                vs = slice(v * 512, (v + 1) * 512)
                for k in range(4):
                    nc.tensor.matmul(ps[:], h2T[k][:, r0:r0 + P],
                                     wfc_sb[k][:, vs],
                                     start=(k == 0),
                                     stop=(k == 3 and _BIAS_ZERO))
                if not _BIAS_ZERO:
                    nc.tensor.matmul(ps[:], ones_r[:, 0:P], bfc_sb[:, vs],
                                     start=False, stop=True)
                ev = lstm_sb.tile([P, 512], f32, tag="fev", bufs=3, name="fev")
                nc.vector.tensor_copy(ev[:], ps[:])
                nc.sync.dma_start(d["logits"][m * P:(m + 1) * P, vs], ev[:])

            h_last = None
            for t in range(T):
                if t == 0:
                    prev = lambda k: h1T[k][:, 7 * B:8 * B]
                else:
                    prev = lambda k, _t=t: h2T[k][:, ((_t - 1) % 8) * B:
                                                  (((_t - 1) % 8) + 1) * B]
                if t >= 5:
                    fc_unit2(2 * (t - 5))
                    fc_unit2(2 * (t - 5) + 1)
                gps_t = lstm_ids(t, z2x_d)
                if h_last is not None:
                    lstm_hT(t - 1, h_last, h2T)
                h_last = lstm_step(t, gps_t, wh2_sb, prev, h2T)

            lstm_hT(T - 1, h_last, h2T)
            for q in range(2 * (T - 5), 8 * (TOK // P)):
                fc_unit2(q)

            nc.sync.dma_start(d["hT"][:, :], h_last[:])
            nc.sync.dma_start(d["cT"][:, :], c_sb[:])


_NC_CACHE = {}
_BIAS_ZERO = True


def _get_program(bias_zero=True):
    key = ("nc", bias_zero)
    if key not in _NC_CACHE:
        _NC_CACHE[key] = build_program(bias_zero)
    return _NC_CACHE[key]


def _stage_inputs(x, salidas_encoder, estado_h, estado_c, emb, W1, b1, W2, b2,
                  Va, bV, Wx1, Wh1, bl1, Wx2, Wh2, bl2, Wfc, bfc):
    f = np.float32
    x = np.asarray(x)
    enc = np.ascontiguousarray(np.asarray(salidas_encoder, f).reshape(STOK, EU))
    h0 = np.ascontiguousarray(np.asarray(estado_h, f))
    c0 = np.ascontiguousarray(np.asarray(estado_c, f))
    emb = np.ascontiguousarray(np.asarray(emb, f))

    # decoder token ids, t-major: ids[t*32+b] = x[b, t]
    ids = np.ascontiguousarray(x.T.reshape(TOK, 1).astype(np.int32))

    # encoder tokens b-major with broadcast h0 rows appended (for u^T GEMM)
    encaugT = np.ascontiguousarray(
        np.concatenate([enc.T, np.repeat(h0, S, axis=0).T], axis=0))

    w12 = np.ascontiguousarray(np.concatenate([np.asarray(W1, f),
                                               np.asarray(W2, f)], axis=0))
    va = np.zeros((P, 8), f)
    va[:, 0::2] = np.asarray(Va, f).reshape(4, P).T
    va = np.ascontiguousarray(va)
    b12 = np.ascontiguousarray(
        (np.asarray(b1, f) + np.asarray(b2, f)).reshape(U, 1))

    wx1b = np.ascontiguousarray(np.concatenate(
        [np.asarray(Wx1, f), np.asarray(bl1, f).reshape(1, G4)], axis=0))
    wx2b = np.ascontiguousarray(np.concatenate(
        [np.asarray(Wx2, f), np.asarray(bl2, f).reshape(1, G4)], axis=0))
    wh1 = np.ascontiguousarray(np.asarray(Wh1, f))
    wh2 = np.ascontiguousarray(np.asarray(Wh2, f))

    common = dict(ids=ids, emb=emb, enc=enc, encaugT=encaugT, w12=w12, va=va,
                  b12=b12, h0=h0, c0=c0, wx1b=wx1b, wh1=wh1, wx2b=wx2b,
                  wh2=wh2)

    wfc = np.asarray(Wfc, f)
    bfc = np.asarray(bfc, f)
    in_maps = []
    for j in range(NC):
        wfcs = np.zeros((U + 1, VSP), f)
        wfcs[:U, :VS] = wfc[:, j * VS:(j + 1) * VS]
        wfcs[U, :VS] = bfc[j * VS:(j + 1) * VS]
        m = dict(common)
        m["wfcb"] = np.ascontiguousarray(wfcs)
        in_maps.append(m)
    return in_maps


def kernel(**inputs):
    bz = all(float(np.abs(np.asarray(inputs[k])).max()) == 0.0
             for k in ("bl1", "bl2", "bfc"))
    nc = _get_program(bias_zero=bz)
    in_maps = _stage_inputs(**inputs)
    res = run_bass_kernel_spmd(nc, in_maps, core_ids=list(range(NC)))
    outs = res.results

    parts = []
    for j in range(NC):
        lg = outs[j]["logits"][:, :VS]  # [2048 (t-major), 4000]
        parts.append(lg.reshape(T, B, VS).transpose(1, 0, 2))
    salidas = np.concatenate(parts, axis=2)

    o0 = outs[0]
    return (salidas, o0["hT"], o0["cT"], o0["pesos"].reshape(B, S, 1))


if __name__ == "__main__":
    print("building program...")
    _get_program()
    print("ok")


# revision 21
# speedup vs baseline: 1.0504x; 1.0504x over previous
"""Trainium2 Bass kernel for the seq2seq decoder (Bahdanau attention + 2-layer
LSTM + vocab projection).

Strategy (8 NeuronCores):
  - Embedding gather, attention, both LSTM recurrences are replicated on every
    core at full batch B=32 (the recurrence streams Wh through the PE every
    step, so its wall time is independent of the batch shard; per-step
    collectives would cost more than they save).
  - The fc [512, 32000] weight is sharded column-wise (vocab) 8 ways: each
    core computes logits for its 4000-vocab slice => zero collectives.
  - Host reassembles: concat logits slices on vocab, takes hT/cT/pesos from
    core 0.

Matmuls run as float32r (full PE rate for moving dim >= 256). Decoder token
order is t-major (tok = t*32 + b) so per-step rows are contiguous.
"""

import os
import sys

for _p in ("/opt/trn_rl_repo",):
    if _p not in sys.path and os.path.isdir(_p):
        sys.path.insert(0, _p)

from contextlib import ExitStack

import numpy as np

import concourse.bass as bass
import concourse.mybir as mybir
import concourse.tile as tile
from concourse import bacc
from concourse.bass_utils import run_bass_kernel_spmd
from concourse.masks import make_identity

B, S, T = 32, 128, 64
VOC, E, U, EU = 32000, 256, 512, 512
NC = 8
VS = VOC // NC  # 4000 vocab per core
VSP = 4096  # padded vocab slice
TOK = B * T  # 2048 decoder tokens
STOK = B * S  # 4096 encoder tokens
G4 = 4 * U  # 2048 gate width
P = 128

f32 = mybir.dt.float32
f32r = mybir.dt.float32r
i32 = mybir.dt.int32
AF = mybir.ActivationFunctionType
AX = mybir.AxisListType

# All matmul operands are staged in SBUF as float32r (walrus requires fp32r
# matmul inputs to be produced/rounded as fp32r, so tiles are typed f32r and
# the producing DMA/ACT op converts on write).


def _r(ap, group):
    return ap


def build_program(bias_zero=True):
    global _BIAS_ZERO
    _BIAS_ZERO = bias_zero
    nc = bacc.Bacc("TRN2", target_bir_lowering=False, debug=False,
                   enable_asserts=False, num_devices=NC)

    d = {}

    def inp(name, shape, dtype=f32):
        d[name] = nc.dram_tensor(name, shape, dtype, kind="ExternalInput").ap()

    def outp(name, shape, dtype=f32):
        d[name] = nc.dram_tensor(name, shape, dtype, kind="ExternalOutput").ap()

    inp("ids", [TOK, 1], i32)
    inp("emb", [VOC, E])
    inp("enc", [STOK, EU], f32r)
    inp("encaugT", [EU + U, STOK], f32r)  # [enc^T ; bcast(h0)^T], tok b-major
    inp("w12", [EU + U, U], f32r)
    inp("va", [P, 8], f32r)  # host-staged [128, (k,pair)] layout
    inp("b12", [U, 1])
    inp("h0", [B, U])
    inp("c0", [B, U])
    inp("wx1b", [E + EU + 1, G4], f32r)
    inp("wh1", [U, G4], f32r)
    inp("wx2b", [U + 1, G4], f32r)
    inp("wh2", [U, G4], f32r)
    inp("wfcb", [U + 1, VSP], f32r)

    outp("logits", [TOK, VSP])
    outp("pesos", [B, S])
    outp("hT", [B, U])
    outp("cT", [B, U])

    with tile.TileContext(nc) as tc:
        with ExitStack() as ctx:
            _build(ctx, tc, nc, d)

    nc.compile()
    return nc


def _build(ctx, tc, nc, d):
    const = ctx.enter_context(tc.tile_pool(name="const", bufs=1))
    ident = const.tile([P, P], f32)
    make_identity(nc, ident)
    ident_r = const.tile([P, P], f32r)
    nc.vector.tensor_copy(ident_r[:], ident[:])
    ones_row = const.tile([1, P], f32)
    nc.any.memset(ones_row[:], 1.0)
    ones_r = const.tile([1, P], f32r)
    nc.vector.tensor_copy(ones_r[:], ones_row[:])
    dummyw = const.tile([P, P], mybir.dt.bfloat16)
    nc.any.memset(dummyw[:], 0.0)

    def pe_keepwarm(n):
        for _ in range(n):
            nc.tensor.ldweights(dummyw[:])

    # small shared PSUM ring (transposes, tiny matmul outputs): 2 banks
    ps_small = ctx.enter_context(tc.tile_pool(name="ps_small", bufs=2, space="PSUM"))

    def psmall():
        return ps_small.tile([P, P], f32, tag="small", name="small")

    # persistent tiles live for the whole kernel
    state = ctx.enter_context(tc.tile_pool(name="state", bufs=1))
    c_sb = state.tile([B, U], f32, tag="c_state")
    nc.sync.dma_start(c_sb[:], d["c0"][:, :])
    h0_sb = state.tile([B, U], f32, tag="h0")
    nc.sync.dma_start(h0_sb[:], d["h0"][:, :])
    h0T = state.tile([P, 4 * B], f32r, tag="h0T")
    # rolling h^T buffers: 8 steps deep (step t -> col block (t % 8))
    h1T = [state.tile([P, 8 * B], f32r, tag=f"h1T{k}", name=f"h1T{k}") for k in range(4)]
    h2T = [state.tile([P, 8 * B], f32r, tag=f"h2T{k}", name=f"h2T{k}") for k in range(4)]
    ctxT = [state.tile([P, B], f32r, tag=f"ctxT{m}", name=f"ctxT{m}") for m in range(4)]

    # h0T: 4 PE transposes of h0 [32,512] -> [128, 4*32]
    for k in range(4):
        tp = psmall()
        nc.tensor.transpose(tp[0:P, 0:B], h0_sb[:, k * P:(k + 1) * P],
                            ident[0:B, 0:B])
        nc.scalar.activation(h0T[:, k * B:(k + 1) * B], tp[0:P, 0:B], AF.Identity)

    bc_pool = tc.tile_pool(name="bc_pool", bufs=1)
    bc = bc_pool.__enter__()
    ctxbc = [bc.tile([P, P], f32r, tag=f"ctxbc{m}", name=f"ctxbc{m}") for m in range(4)]
    xeT = [bc.tile([P, TOK], f32r, tag=f"xeT{c}", name=f"xeT{c}") for c in range(2)]
    with tc.tile_pool(name="embp", bufs=3) as embp:
        for i in range(TOK // P):
            idx = embp.tile([P, 1], i32, tag="idx")
            nc.sync.dma_start(idx[:], d["ids"][i * P:(i + 1) * P, :])
            xe = embp.tile([P, E], f32, tag="xe")
            nc.gpsimd.indirect_dma_start(
                out=xe[:], out_offset=None, in_=d["emb"][:, :],
                in_offset=bass.IndirectOffsetOnAxis(ap=idx[:, 0:1], axis=0))
            for c in range(2):
                tp = psmall()
                nc.tensor.transpose(tp[:], xe[:, c * P:(c + 1) * P], ident[:])
                nc.scalar.activation(xeT[c][:, i * P:(i + 1) * P], tp[:], AF.Identity)


    # ---------------- Phase A: attention ----------------
    with tc.tile_pool(name="attn", bufs=1) as attn, \
         tc.tile_pool(name="attn_mov", bufs=2) as attn_mov, \
         tc.tile_pool(name="up_ps", bufs=2, space="PSUM") as up_ps, \
         tc.tile_pool(name="ctx_ps", bufs=1, space="PSUM") as ctx_ps:

        w12_sb = [attn.tile([P, U], f32r, tag=f"w12_{k}", name=f"w12_{k}") for k in range(8)]
        for k in range(8):
            nc.sync.dma_start(w12_sb[k][:], d["w12"][k * P:(k + 1) * P, :])
        va_sb = attn.tile([P, 8], f32r, tag="va")
        nc.sync.dma_start(va_sb[:], d["va"][:, :])
        b12_sb = attn.tile([P, 4], f32, tag="b12")
        nc.sync.dma_start(b12_sb[:], d["b12"][:, 0:1].rearrange("(k p) o -> p (k o)", p=P))

        tanhuT = [attn.tile([P, STOK], f32r, tag=f"tanhuT{m}", name=f"tanhuT{m}") for m in range(4)]

        # u^T GEMM: [512 u, 4096 tok] = w12^T-chunks @ encaugT, + tanh fused
        for n in range(8):
            mov = [attn_mov.tile([P, 512], f32r, tag=f"amov{k}", name=f"amov{k}") for k in range(8)]
            for k in range(8):
                nc.sync.dma_start(mov[k][:], d["encaugT"][k * P:(k + 1) * P,
                                                          n * 512:(n + 1) * 512])
            for m in range(4):
                ps = up_ps.tile([P, 512], f32, tag="ups")
                for k in range(8):
                    nc.tensor.matmul(ps[:], _r(w12_sb[k][:, m * P:(m + 1) * P], "attn"),
                                     _r(mov[k][:], "attn"),
                                     start=(k == 0), stop=(k == 7))
                nc.scalar.activation(tanhuT[m][:, n * 512:(n + 1) * 512], ps[:],
                                     AF.Tanh, bias=b12_sb[:, m:m + 1])

        # score^T [128 s, 32 b]
        scoreT = attn.tile([P, B], f32, tag="scoreT")
        for b in range(B):
            ps = psmall()
            for k in range(4):
                nc.tensor.matmul(ps[0:P, 0:2],
                                 _r(tanhuT[k][:, b * P:(b + 1) * P], "attn"),
                                 _r(va_sb[:, 2 * k:2 * k + 2], "attn"),
                                 start=(k == 0), stop=(k == 3))
            nc.scalar.activation(scoreT[:, b:b + 1], ps[0:P, 0:1], AF.Identity)

        # softmax over s: transpose to [32 b, 128 s]
        ps2 = psmall()
        nc.tensor.transpose(ps2[0:B, 0:P], scoreT[:], ident[:])
        sc = attn.tile([B, P], f32, tag="sc")
        nc.scalar.activation(sc[:], ps2[0:B, 0:P], AF.Identity)
        mx = attn.tile([B, 1], f32, tag="mx")
        nc.vector.reduce_max(mx[:], sc[:], axis=AX.X)
        negmx = attn.tile([B, 1], f32, tag="negmx")
        nc.vector.tensor_scalar_mul(negmx[:], mx[:], -1.0)
        pes = attn.tile([B, P], f32, tag="pes")
        sm = attn.tile([B, 1], f32, tag="sm")
        nc.scalar.activation(pes[:], sc[:], AF.Exp, bias=negmx[:, 0:1],
                             accum_out=sm[:, 0:1])
        rs = attn.tile([B, 1], f32, tag="rs")
        nc.vector.reciprocal(rs[:], sm[:])
        pesos_sb = attn.tile([B, P], f32, tag="pesos")
        nc.vector.tensor_scalar_mul(pesos_sb[:], pes[:], rs[:, 0:1])
        nc.sync.dma_start(d["pesos"][:, :], pesos_sb[:])

        # pesos^T [128 s, 32 b]
        ps3 = psmall()
        nc.tensor.transpose(ps3[0:P, 0:B], pesos_sb[:], ident[0:B, 0:B])
        pesT = attn.tile([P, B + 2], f32r, tag="pesT")
        nc.scalar.activation(pesT[:, 0:B], ps3[0:P, 0:B], AF.Identity)
        # keep the overhang pair finite (read, never used)
        nc.scalar.activation(pesT[:, B:B + 2], ps3[0:P, 0:2], AF.Identity)

        # ctx^T [512 e, 32 b]: per-b matvecs against native enc[b] tiles
        # psum columns strided by 2 to keep matmul outputs 8-byte aligned
        ctxps = [ctx_ps.tile([P, 2 * B], f32, tag=f"ctxps{m}", name=f"ctxps{m}") for m in range(4)]
        for b in range(B):
            encb = attn_mov.tile([P, EU], f32r, tag="encb")
            nc.sync.dma_start(encb[:], d["enc"][b * P:(b + 1) * P, :])
            for m in range(4):
                nc.tensor.matmul(ctxps[m][:, 2 * b:2 * b + 2],
                                 _r(encb[:, m * P:(m + 1) * P], "attn"),
                                 _r(pesT[:, b:b + 2], "attn"),
                                 start=True, stop=True)
        for m in range(4):
            nc.scalar.activation(ctxT[m][:], ctxps[m][:, 0:2 * B:2], AF.Identity)

    # ctx broadcast tiles [128, 128]: col (t%4)*32+b = ctxT[:, b]
    for m in range(4):
        for j in range(4):
            nc.vector.tensor_copy(ctxbc[m][:, j * B:(j + 1) * B], ctxT[m][:])

    # ---------------- Phase C: z1x = [xe|ctx|1] @ [Wx1; bl1] -> DRAM ------
    dram = ctx.enter_context(tc.tile_pool(name="dram", bufs=1, space="DRAM"))
    z1x_d = dram.tile([TOK, G4], f32r, tag="z1x")
    z2x_d = dram.tile([TOK, G4], f32r, tag="z2x")

    with tc.tile_pool(name="z1xw", bufs=1) as z1xw, \
         tc.tile_pool(name="z1xev", bufs=4) as z1xev, \
         tc.tile_pool(name="z1xps", bufs=4, space="PSUM") as z1xps:
        wx1_sb = [z1xw.tile([P, G4], f32r, tag=f"wx1_{k}", name=f"wx1_{k}") for k in range(6)]
        for k in range(6):
            nc.sync.dma_start(wx1_sb[k][:], d["wx1b"][k * P:(k + 1) * P, :])
        bl1_sb = z1xw.tile([1, G4], f32r, tag="bl1")
        nc.sync.dma_start(bl1_sb[:], d["wx1b"][E + EU:E + EU + 1, :])

        for m in range(TOK // P):
            for g in range(4):
                ps = z1xps.tile([P, 512], f32, tag="zps")
                gs = slice(g * 512, (g + 1) * 512)
                for k in range(2):
                    nc.tensor.matmul(ps[:], _r(xeT[k][:, m * P:(m + 1) * P], "zx"),
                                     _r(wx1_sb[k][:, gs], "zx"),
                                     start=(k == 0), stop=False)
                for k in range(4):
                    nc.tensor.matmul(ps[:], _r(ctxbc[k][:], "zx"),
                                     _r(wx1_sb[2 + k][:, gs], "zx"),
                                     start=False, stop=(k == 3 and _BIAS_ZERO))
                if not _BIAS_ZERO:
                    nc.tensor.matmul(ps[:], _r(ones_r[:, 0:P], "zx"),
                                     _r(bl1_sb[:, gs], "zx"), start=False,
                                     stop=True)
                ev = z1xev.tile([P, 512], f32r, tag="zev")
                nc.vector.tensor_copy(ev[:], ps[:])
                nc.sync.dma_start(z1x_d[m * P:(m + 1) * P, gs], ev[:])

    bc_pool.__exit__(None, None, None)

    # ---------------- Phase D/E: LSTM layers with interleaved GEMMs -------
    with tc.tile_pool(name="lstm_sb", bufs=1) as lstm_sb, \
         tc.tile_pool(name="lstm_zx", bufs=3) as lstm_zx, \
         tc.tile_pool(name="lstm_ps", bufs=1, space="PSUM") as lstm_ps, \
         tc.tile_pool(name="lstm_tp", bufs=2, space="PSUM") as lstm_tp, \
         tc.tile_pool(name="w2pool", bufs=1) as w2pool:

        # prefetch wh2 early (free DMA time during layer 1)
        wh2_sb = [w2pool.tile([P, G4], f32r, tag=f"wh2_{k}", name=f"wh2_{k}")
                  for k in range(4)]
        for k in range(4):
            nc.sync.dma_start(wh2_sb[k][:], d["wh2"][k * P:(k + 1) * P, :])

        def lstm_ids(t, zx_dram):
            """Emit the z-identity matmuls (no h^T dependency)."""
            zx = lstm_zx.tile([B, G4], f32r, tag="zx", name="zx")
            nc.sync.dma_start(zx[:], zx_dram[t * B:(t + 1) * B, :])
            gps = {}
            for g in (1, 0, 2, 3):
                ps = lstm_ps.tile([B, 512], f32, tag=f"gate{g}", name=f"gate{g}")
                gs = slice(g * 512, (g + 1) * 512)
                nc.tensor.matmul(ps[:], ident_r[0:B, 0:B], zx[:, gs],
                                 start=True, stop=False)
                gps[g] = ps
            return gps

        def lstm_step(t, gps, wh_sb, hT_prev_ap, hT_out):
            """Wh matmuls + cell; h^T transposes emitted separately."""
            for g in (1, 0, 2, 3):
                ps = gps[g]
                gs = slice(g * 512, (g + 1) * 512)
                for k in range(4):
                    nc.tensor.matmul(ps[:], hT_prev_ap(k), wh_sb[k][:, gs],
                                     start=False, stop=(k == 3))
            gi = lstm_sb.tile([B, 512], f32, tag="gi")
            gf = lstm_sb.tile([B, 512], f32, tag="gf")
            gg = lstm_sb.tile([B, 512], f32, tag="gg")
            go = lstm_sb.tile([B, 512], f32, tag="go")
            t1 = lstm_sb.tile([B, 512], f32, tag="t1")
            t2 = lstm_sb.tile([B, 512], f32, tag="t2")
            tcl = lstm_sb.tile([B, 512], f32, tag="tc")
            h = lstm_sb.tile([B, 512], f32, tag="h", bufs=2)
            r = (t % 8) * B
            nc.scalar.activation(gf[:], gps[1][:], AF.Sigmoid)
            nc.scalar.activation(gi[:], gps[0][:], AF.Sigmoid)
            nc.scalar.activation(gg[:], gps[2][:], AF.Tanh)
            nc.scalar.activation(go[:], gps[3][:], AF.Sigmoid)
            del gps
            nc.vector.tensor_mul(t1[:], gf[:], c_sb[:])
            nc.vector.tensor_mul(t2[:], gi[:], gg[:])
            nc.vector.tensor_add(c_sb[:], t1[:], t2[:])
            nc.scalar.activation(tcl[:], c_sb[:], AF.Tanh)
            nc.vector.tensor_mul(h[:], go[:], tcl[:])
            return h

        def lstm_hT(t, h, hT_out):
            r = (t % 8) * B
            for k in range(4):
                tp = lstm_tp.tile([P, B], f32, tag="htp", name="htp")
                nc.tensor.transpose(tp[:], h[:, k * P:(k + 1) * P],
                                    ident[0:B, 0:B])
                nc.scalar.activation(hT_out[k][:, r:r + B], tp[:], AF.Identity)

        # ---- layer 1, with z2x chunks interleaved every 4 steps ----
        with tc.tile_pool(name="wpool", bufs=1) as wpool:
            wh1_sb = [wpool.tile([P, G4], f32r, tag=f"wh1_{k}", name=f"wh1_{k}")
                      for k in range(4)]
            wx2_sb = [wpool.tile([P, G4], f32r, tag=f"wx2_{k}", name=f"wx2_{k}")
                      for k in range(4)]
            for k in range(4):
                nc.sync.dma_start(wh1_sb[k][:], d["wh1"][k * P:(k + 1) * P, :])
                nc.sync.dma_start(wx2_sb[k][:], d["wx2b"][k * P:(k + 1) * P, :])
            bl2_sb = wpool.tile([1, G4], f32r, tag="bl2")
            nc.sync.dma_start(bl2_sb[:], d["wx2b"][U:U + 1, :])

            def z2x_unit_l1(q):
                # one gate-chunk of z2x token-chunk m (steps 4m..4m+3)
                m, g = q // 4, q % 4
                r0 = ((4 * m) % 8) * B
                ps = ps_small.tile([P, 512], f32, tag="small", name=f"z2ps{g}")
                gs = slice(g * 512, (g + 1) * 512)
                for k in range(4):
                    nc.tensor.matmul(ps[:], h1T[k][:, r0:r0 + P],
                                     wx2_sb[k][:, gs],
                                     start=(k == 0),
                                     stop=(k == 3 and _BIAS_ZERO))
                if not _BIAS_ZERO:
                    nc.tensor.matmul(ps[:], ones_r[:, 0:P], bl2_sb[:, gs],
                                     start=False, stop=True)
                ev = lstm_sb.tile([P, 512], f32r, tag="z2ev", bufs=2, name="z2ev")
                nc.vector.tensor_copy(ev[:], ps[:])
                nc.sync.dma_start(z2x_d[m * P:(m + 1) * P, gs], ev[:])

            h_prev_l1 = None
            for t in range(T):
                if t == 0:
                    prev = lambda k: h0T[:, k * B:(k + 1) * B]
                else:
                    prev = lambda k, _t=t: h1T[k][:, ((_t - 1) % 8) * B:
                                                  (((_t - 1) % 8) + 1) * B]
                gps_t = lstm_ids(t, z1x_d)
                h_t = lstm_step(t, gps_t, wh1_sb, prev, h1T)
                if t >= 4:
                    z2x_unit_l1(t - 4)
                lstm_hT(t, h_t, h1T)
                h_prev_l1 = h_t

            for q in range(T - 4, T):
                z2x_unit_l1(q)

        # ---- layer 2, with FC chunks interleaved every 4 steps ----
        with tc.tile_pool(name="fcw", bufs=1) as fcw:
            wfc_sb = [fcw.tile([P, VSP], f32r, tag=f"wfc_{k}", name=f"wfc_{k}")
                      for k in range(4)]
            for k in range(4):
                nc.sync.dma_start(wfc_sb[k][:], d["wfcb"][k * P:(k + 1) * P, :])
            bfc_sb = fcw.tile([1, VSP], f32r, tag="bfc")
            nc.sync.dma_start(bfc_sb[:], d["wfcb"][U:U + 1, :])

            def fc_unit2(q):
                # one vocab-chunk of logits token-chunk m (steps 4m..4m+3)
                m, v = q // 8, q % 8
                r0 = ((4 * m) % 8) * B
                ps = ps_small.tile([P, 512], f32, tag="small", name="fps")
                vs = slice(v * 512, (v + 1) * 512)
                for k in range(4):
                    nc.tensor.matmul(ps[:], h2T[k][:, r0:r0 + P],
                                     wfc_sb[k][:, vs],
                                     start=(k == 0),
                                     stop=(k == 3 and _BIAS_ZERO))
                if not _BIAS_ZERO:
                    nc.tensor.matmul(ps[:], ones_r[:, 0:P], bfc_sb[:, vs],
                                     start=False, stop=True)
                ev = lstm_sb.tile([P, 512], f32, tag="fev", bufs=3, name="fev")
                nc.vector.tensor_copy(ev[:], ps[:])
                nc.sync.dma_start(d["logits"][m * P:(m + 1) * P, vs], ev[:])

            h_last = None
            for t in range(T):
                if t == 0:
                    prev = lambda k: h1T[k][:, 7 * B:8 * B]
                else:
                    prev = lambda k, _t=t: h2T[k][:, ((_t - 1) % 8) * B:
                                                  (((_t - 1) % 8) + 1) * B]
                gps_t = lstm_ids(t, z2x_d)
                h_last = lstm_step(t, gps_t, wh2_sb, prev, h2T)
                if t >= 4:
                    fc_unit2(2 * (t - 4))
                    fc_unit2(2 * (t - 4) + 1)
                lstm_hT(t, h_last, h2T)

            for q in range(2 * (T - 4), 8 * (TOK // P)):
                fc_unit2(q)

            nc.sync.dma_start(d["hT"][:, :], h_last[:])
            nc.sync.dma_start(d["cT"][:, :], c_sb[:])


_NC_CACHE = {}
_BIAS_ZERO = True


def _get_program(bias_zero=True):
    key = ("nc", bias_zero)
    if key not in _NC_CACHE:
        _NC_CACHE[key] = build_program(bias_zero)
    return _NC_CACHE[key]


def _stage_inputs(x, salidas_encoder, estado_h, estado_c, emb, W1, b1, W2, b2,
                  Va, bV, Wx1, Wh1, bl1, Wx2, Wh2, bl2, Wfc, bfc):
    f = np.float32
    x = np.asarray(x)
    enc = np.ascontiguousarray(np.asarray(salidas_encoder, f).reshape(STOK, EU))
    h0 = np.ascontiguousarray(np.asarray(estado_h, f))
    c0 = np.ascontiguousarray(np.asarray(estado_c, f))
    emb = np.ascontiguousarray(np.asarray(emb, f))

    # decoder token ids, t-major: ids[t*32+b] = x[b, t]
    ids = np.ascontiguousarray(x.T.reshape(TOK, 1).astype(np.int32))

    # encoder tokens b-major with broadcast h0 rows appended (for u^T GEMM)
    encaugT = np.ascontiguousarray(
        np.concatenate([enc.T, np.repeat(h0, S, axis=0).T], axis=0))

    w12 = np.ascontiguousarray(np.concatenate([np.asarray(W1, f),
                                               np.asarray(W2, f)], axis=0))
    va = np.zeros((P, 8), f)
    va[:, 0::2] = np.asarray(Va, f).reshape(4, P).T
    va = np.ascontiguousarray(va)
    b12 = np.ascontiguousarray(
        (np.asarray(b1, f) + np.asarray(b2, f)).reshape(U, 1))

    wx1b = np.ascontiguousarray(np.concatenate(
        [np.asarray(Wx1, f), np.asarray(bl1, f).reshape(1, G4)], axis=0))
    wx2b = np.ascontiguousarray(np.concatenate(
        [np.asarray(Wx2, f), np.asarray(bl2, f).reshape(1, G4)], axis=0))
    wh1 = np.ascontiguousarray(np.asarray(Wh1, f))
    wh2 = np.ascontiguousarray(np.asarray(Wh2, f))

    common = dict(ids=ids, emb=emb, enc=enc, encaugT=encaugT, w12=w12, va=va,
                  b12=b12, h0=h0, c0=c0, wx1b=wx1b, wh1=wh1, wx2b=wx2b,
                  wh2=wh2)

    wfc = np.asarray(Wfc, f)
    bfc = np.asarray(bfc, f)
    in_maps = []
    for j in range(NC):
        wfcs = np.zeros((U + 1, VSP), f)
        wfcs[:U, :VS] = wfc[:, j * VS:(j + 1) * VS]
        wfcs[U, :VS] = bfc[j * VS:(j + 1) * VS]
        m = dict(common)
        m["wfcb"] = np.ascontiguousarray(wfcs)
        in_maps.append(m)
    return in_maps


def kernel(**inputs):
    bz = all(float(np.abs(np.asarray(inputs[k])).max()) == 0.0
             for k in ("bl1", "bl2", "bfc"))
    nc = _get_program(bias_zero=bz)
    in_maps = _stage_inputs(**inputs)
    res = run_bass_kernel_spmd(nc, in_maps, core_ids=list(range(NC)))
    outs = res.results

    parts = []
    for j in range(NC):
        lg = outs[j]["logits"][:, :VS]  # [2048 (t-major), 4000]
        parts.append(lg.reshape(T, B, VS).transpose(1, 0, 2))
    salidas = np.concatenate(parts, axis=2)

    o0 = outs[0]
    return (salidas, o0["hT"], o0["cT"], o0["pesos"].reshape(B, S, 1))


if __name__ == "__main__":
    print("building program...")
    _get_program()
    print("ok")


# revision 22
# speedup vs baseline: 1.1074x; 1.0543x over previous
"""Trainium2 Bass kernel for the seq2seq decoder (Bahdanau attention + 2-layer
LSTM + vocab projection).

Strategy (8 NeuronCores):
  - Embedding gather, attention, both LSTM recurrences are replicated on every
    core at full batch B=32 (the recurrence streams Wh through the PE every
    step, so its wall time is independent of the batch shard; per-step
    collectives would cost more than they save).
  - The fc [512, 32000] weight is sharded column-wise (vocab) 8 ways: each
    core computes logits for its 4000-vocab slice => zero collectives.
  - Host reassembles: concat logits slices on vocab, takes hT/cT/pesos from
    core 0.

Matmuls run as float32r (full PE rate for moving dim >= 256). Decoder token
order is t-major (tok = t*32 + b) so per-step rows are contiguous.
"""

import os
import sys

for _p in ("/opt/trn_rl_repo",):
    if _p not in sys.path and os.path.isdir(_p):
        sys.path.insert(0, _p)

from contextlib import ExitStack

import numpy as np

import concourse.bass as bass
import concourse.mybir as mybir
import concourse.tile as tile
from concourse import bacc
from concourse.bass_utils import run_bass_kernel_spmd
from concourse.masks import make_identity

B, S, T = 32, 128, 64
VOC, E, U, EU = 32000, 256, 512, 512
NC = 8
VS = VOC // NC  # 4000 vocab per core
VSP = 4096  # padded vocab slice
TOK = B * T  # 2048 decoder tokens
STOK = B * S  # 4096 encoder tokens
G4 = 4 * U  # 2048 gate width
P = 128

f32 = mybir.dt.float32
f32r = mybir.dt.float32r
i32 = mybir.dt.int32
AF = mybir.ActivationFunctionType
AX = mybir.AxisListType

# All matmul operands are staged in SBUF as float32r (walrus requires fp32r
# matmul inputs to be produced/rounded as fp32r, so tiles are typed f32r and
# the producing DMA/ACT op converts on write).


def _r(ap, group):
    return ap


def build_program(bias_zero=True):
    global _BIAS_ZERO
    _BIAS_ZERO = bias_zero
    nc = bacc.Bacc("TRN2", target_bir_lowering=False, debug=False,
                   enable_asserts=False, num_devices=NC)

    d = {}

    def inp(name, shape, dtype=f32):
        d[name] = nc.dram_tensor(name, shape, dtype, kind="ExternalInput").ap()

    def outp(name, shape, dtype=f32):
        d[name] = nc.dram_tensor(name, shape, dtype, kind="ExternalOutput").ap()

    inp("ids", [TOK, 1], i32)
    inp("emb", [VOC, E])
    inp("enc", [STOK, EU], f32r)
    inp("encaugT", [EU + U, STOK], f32r)  # [enc^T ; bcast(h0)^T], tok b-major
    inp("w12", [EU + U, U], f32r)
    inp("va", [P, 8], f32r)  # host-staged [128, (k,pair)] layout
    inp("b12", [U, 1])
    inp("h0", [B, U])
    inp("c0", [B, U])
    inp("wx1b", [E + EU + 1, G4], f32r)
    inp("wh1", [U, G4], f32r)
    inp("wx2b", [U + 1, G4], f32r)
    inp("wh2", [U, G4], f32r)
    inp("wfcb", [U + 1, VSP], f32r)

    outp("logits", [TOK, VSP])
    outp("pesos", [B, S])
    outp("hT", [B, U])
    outp("cT", [B, U])

    with tile.TileContext(nc) as tc:
        with ExitStack() as ctx:
            _build(ctx, tc, nc, d)

    nc.compile()
    return nc


def _build(ctx, tc, nc, d):
    const = ctx.enter_context(tc.tile_pool(name="const", bufs=1))
    ident = const.tile([P, P], f32)
    make_identity(nc, ident)
    ident_r = const.tile([P, P], f32r)
    nc.vector.tensor_copy(ident_r[:], ident[:])
    ones_row = const.tile([1, P], f32)
    nc.any.memset(ones_row[:], 1.0)
    ones_r = const.tile([1, P], f32r)
    nc.vector.tensor_copy(ones_r[:], ones_row[:])
    dummyw = const.tile([P, P], mybir.dt.bfloat16)
    nc.any.memset(dummyw[:], 0.0)

    def pe_keepwarm(n):
        for _ in range(n):
            nc.tensor.ldweights(dummyw[:])

    # small shared PSUM ring (transposes, tiny matmul outputs): 2 banks
    ps_small = ctx.enter_context(tc.tile_pool(name="ps_small", bufs=2, space="PSUM"))

    def psmall():
        return ps_small.tile([P, P], f32, tag="small", name="small")

    # persistent tiles live for the whole kernel
    state = ctx.enter_context(tc.tile_pool(name="state", bufs=1))
    c_sb = state.tile([B, U], f32, tag="c_state")
    nc.sync.dma_start(c_sb[:], d["c0"][:, :])
    h0_sb = state.tile([B, U], f32, tag="h0")
    nc.sync.dma_start(h0_sb[:], d["h0"][:, :])
    h0T = state.tile([P, 4 * B], f32r, tag="h0T")
    # rolling h^T buffers: 8 steps deep (step t -> col block (t % 8))
    h1T = [state.tile([P, 8 * B], f32r, tag=f"h1T{k}", name=f"h1T{k}") for k in range(4)]
    h2T = [state.tile([P, 8 * B], f32r, tag=f"h2T{k}", name=f"h2T{k}") for k in range(4)]
    ctxT = [state.tile([P, B], f32r, tag=f"ctxT{m}", name=f"ctxT{m}") for m in range(4)]

    # h0T: 4 PE transposes of h0 [32,512] -> [128, 4*32]
    for k in range(4):
        tp = psmall()
        nc.tensor.transpose(tp[0:P, 0:B], h0_sb[:, k * P:(k + 1) * P],
                            ident[0:B, 0:B])
        nc.scalar.activation(h0T[:, k * B:(k + 1) * B], tp[0:P, 0:B], AF.Identity)

    bc_pool = tc.tile_pool(name="bc_pool", bufs=1)
    bc = bc_pool.__enter__()
    ctxbc = [bc.tile([P, P], f32r, tag=f"ctxbc{m}", name=f"ctxbc{m}") for m in range(4)]
    xeT = [bc.tile([P, TOK], f32r, tag=f"xeT{c}", name=f"xeT{c}") for c in range(2)]
    with tc.tile_pool(name="embp", bufs=3) as embp:
        for i in range(TOK // P):
            idx = embp.tile([P, 1], i32, tag="idx")
            nc.sync.dma_start(idx[:], d["ids"][i * P:(i + 1) * P, :])
            xe = embp.tile([P, E], f32, tag="xe")
            nc.gpsimd.indirect_dma_start(
                out=xe[:], out_offset=None, in_=d["emb"][:, :],
                in_offset=bass.IndirectOffsetOnAxis(ap=idx[:, 0:1], axis=0))
            for c in range(2):
                tp = psmall()
                nc.tensor.transpose(tp[:], xe[:, c * P:(c + 1) * P], ident[:])
                nc.scalar.activation(xeT[c][:, i * P:(i + 1) * P], tp[:], AF.Identity)


    # ---------------- Phase A: attention ----------------
    with tc.tile_pool(name="attn", bufs=1) as attn, \
         tc.tile_pool(name="attn_mov", bufs=2) as attn_mov, \
         tc.tile_pool(name="up_ps", bufs=2, space="PSUM") as up_ps, \
         tc.tile_pool(name="ctx_ps", bufs=1, space="PSUM") as ctx_ps:

        w12_sb = [attn.tile([P, U], f32r, tag=f"w12_{k}", name=f"w12_{k}") for k in range(8)]
        for k in range(8):
            nc.sync.dma_start(w12_sb[k][:], d["w12"][k * P:(k + 1) * P, :])
        va_sb = attn.tile([P, 8], f32r, tag="va")
        nc.sync.dma_start(va_sb[:], d["va"][:, :])
        b12_sb = attn.tile([P, 4], f32, tag="b12")
        nc.sync.dma_start(b12_sb[:], d["b12"][:, 0:1].rearrange("(k p) o -> p (k o)", p=P))

        tanhuT = [attn.tile([P, STOK], f32r, tag=f"tanhuT{m}", name=f"tanhuT{m}") for m in range(4)]

        # u^T GEMM: [512 u, 4096 tok] = w12^T-chunks @ encaugT, + tanh fused
        for n in range(8):
            mov = [attn_mov.tile([P, 512], f32r, tag=f"amov{k}", name=f"amov{k}") for k in range(8)]
            for k in range(8):
                nc.sync.dma_start(mov[k][:], d["encaugT"][k * P:(k + 1) * P,
                                                          n * 512:(n + 1) * 512])
            for m in range(4):
                ps = up_ps.tile([P, 512], f32, tag="ups")
                for k in range(8):
                    nc.tensor.matmul(ps[:], _r(w12_sb[k][:, m * P:(m + 1) * P], "attn"),
                                     _r(mov[k][:], "attn"),
                                     start=(k == 0), stop=(k == 7))
                nc.scalar.activation(tanhuT[m][:, n * 512:(n + 1) * 512], ps[:],
                                     AF.Tanh, bias=b12_sb[:, m:m + 1])

        # score^T [128 s, 32 b]
        scoreT = attn.tile([P, B], f32, tag="scoreT")
        for b in range(B):
            ps = psmall()
            for k in range(4):
                nc.tensor.matmul(ps[0:P, 0:2],
                                 _r(tanhuT[k][:, b * P:(b + 1) * P], "attn"),
                                 _r(va_sb[:, 2 * k:2 * k + 2], "attn"),
                                 start=(k == 0), stop=(k == 3))
            nc.scalar.activation(scoreT[:, b:b + 1], ps[0:P, 0:1], AF.Identity)

        # softmax over s: transpose to [32 b, 128 s]
        ps2 = psmall()
        nc.tensor.transpose(ps2[0:B, 0:P], scoreT[:], ident[:])
        sc = attn.tile([B, P], f32, tag="sc")
        nc.scalar.activation(sc[:], ps2[0:B, 0:P], AF.Identity)
        mx = attn.tile([B, 1], f32, tag="mx")
        nc.vector.reduce_max(mx[:], sc[:], axis=AX.X)
        negmx = attn.tile([B, 1], f32, tag="negmx")
        nc.vector.tensor_scalar_mul(negmx[:], mx[:], -1.0)
        pes = attn.tile([B, P], f32, tag="pes")
        sm = attn.tile([B, 1], f32, tag="sm")
        nc.scalar.activation(pes[:], sc[:], AF.Exp, bias=negmx[:, 0:1],
                             accum_out=sm[:, 0:1])
        rs = attn.tile([B, 1], f32, tag="rs")
        nc.vector.reciprocal(rs[:], sm[:])
        pesos_sb = attn.tile([B, P], f32, tag="pesos")
        nc.vector.tensor_scalar_mul(pesos_sb[:], pes[:], rs[:, 0:1])
        nc.sync.dma_start(d["pesos"][:, :], pesos_sb[:])

        # pesos^T [128 s, 32 b]
        ps3 = psmall()
        nc.tensor.transpose(ps3[0:P, 0:B], pesos_sb[:], ident[0:B, 0:B])
        pesT = attn.tile([P, B + 2], f32r, tag="pesT")
        nc.scalar.activation(pesT[:, 0:B], ps3[0:P, 0:B], AF.Identity)
        # keep the overhang pair finite (read, never used)
        nc.scalar.activation(pesT[:, B:B + 2], ps3[0:P, 0:2], AF.Identity)

        # ctx^T [512 e, 32 b]: per-b matvecs against native enc[b] tiles
        # psum columns strided by 2 to keep matmul outputs 8-byte aligned
        ctxps = [ctx_ps.tile([P, 2 * B], f32, tag=f"ctxps{m}", name=f"ctxps{m}") for m in range(4)]
        for b in range(B):
            encb = attn_mov.tile([P, EU], f32r, tag="encb")
            nc.sync.dma_start(encb[:], d["enc"][b * P:(b + 1) * P, :])
            for m in range(4):
                nc.tensor.matmul(ctxps[m][:, 2 * b:2 * b + 2],
                                 _r(encb[:, m * P:(m + 1) * P], "attn"),
                                 _r(pesT[:, b:b + 2], "attn"),
                                 start=True, stop=True)
        for m in range(4):
            nc.scalar.activation(ctxT[m][:], ctxps[m][:, 0:2 * B:2], AF.Identity)

    # ctx broadcast tiles [128, 128]: col (t%4)*32+b = ctxT[:, b]
    for m in range(4):
        for j in range(4):
            nc.vector.tensor_copy(ctxbc[m][:, j * B:(j + 1) * B], ctxT[m][:])

    # ---------------- Phase C: z1x = [xe|ctx|1] @ [Wx1; bl1] -> DRAM ------
    dram = ctx.enter_context(tc.tile_pool(name="dram", bufs=1, space="DRAM"))
    z1x_d = dram.tile([TOK, G4], f32r, tag="z1x")
    z2x_d = dram.tile([TOK, G4], f32r, tag="z2x")

    with tc.tile_pool(name="z1xw", bufs=1) as z1xw, \
         tc.tile_pool(name="z1xev", bufs=4) as z1xev, \
         tc.tile_pool(name="z1xps", bufs=4, space="PSUM") as z1xps:
        wx1_sb = [z1xw.tile([P, G4], f32r, tag=f"wx1_{k}", name=f"wx1_{k}") for k in range(6)]
        for k in range(6):
            nc.sync.dma_start(wx1_sb[k][:], d["wx1b"][k * P:(k + 1) * P, :])
        bl1_sb = z1xw.tile([1, G4], f32r, tag="bl1")
        nc.sync.dma_start(bl1_sb[:], d["wx1b"][E + EU:E + EU + 1, :])

        for m in range(TOK // P):
            for g in range(4):
                ps = z1xps.tile([P, 512], f32, tag="zps")
                gs = slice(g * 512, (g + 1) * 512)
                for k in range(2):
                    nc.tensor.matmul(ps[:], _r(xeT[k][:, m * P:(m + 1) * P], "zx"),
                                     _r(wx1_sb[k][:, gs], "zx"),
                                     start=(k == 0), stop=False)
                for k in range(4):
                    nc.tensor.matmul(ps[:], _r(ctxbc[k][:], "zx"),
                                     _r(wx1_sb[2 + k][:, gs], "zx"),
                                     start=False, stop=(k == 3 and _BIAS_ZERO))
                if not _BIAS_ZERO:
                    nc.tensor.matmul(ps[:], _r(ones_r[:, 0:P], "zx"),
                                     _r(bl1_sb[:, gs], "zx"), start=False,
                                     stop=True)
                ev = z1xev.tile([P, 512], f32r, tag="zev")
                nc.vector.tensor_copy(ev[:], ps[:])
                nc.sync.dma_start(z1x_d[m * P:(m + 1) * P, gs], ev[:])

    bc_pool.__exit__(None, None, None)

    # ---------------- Phase D/E: LSTM layers with interleaved GEMMs -------
    with tc.tile_pool(name="lstm_sb", bufs=1) as lstm_sb, \
         tc.tile_pool(name="lstm_zx", bufs=3) as lstm_zx, \
         tc.tile_pool(name="lstm_ps", bufs=1, space="PSUM") as lstm_ps, \
         tc.tile_pool(name="lstm_tp", bufs=2, space="PSUM") as lstm_tp, \
         tc.tile_pool(name="w2pool", bufs=1) as w2pool:

        # prefetch wh2 early (free DMA time during layer 1)
        wh2_sb = [w2pool.tile([P, G4], f32r, tag=f"wh2_{k}", name=f"wh2_{k}")
                  for k in range(4)]
        for k in range(4):
            nc.sync.dma_start(wh2_sb[k][:], d["wh2"][k * P:(k + 1) * P, :])

        def lstm_step(t, zx_dram, wh_sb, hT_prev_ap, hT_out):
            """One LSTM step (gate matmuls + cell); h^T emitted separately."""
            zx = lstm_zx.tile([B, G4], f32r, tag="zx", name="zx")
            nc.sync.dma_start(zx[:], zx_dram[t * B:(t + 1) * B, :])
            gps = {}
            for g in range(4):
                ps = lstm_ps.tile([B, 512], f32, tag=f"gate{g}", name=f"gate{g}")
                gs = slice(g * 512, (g + 1) * 512)
                nc.tensor.matmul(ps[:], ident_r[0:B, 0:B], zx[:, gs],
                                 start=True, stop=False)
                for k in range(4):
                    nc.tensor.matmul(ps[:], hT_prev_ap(k), wh_sb[k][:, gs],
                                     start=False, stop=(k == 3))
                gps[g] = ps
            gi = lstm_sb.tile([B, 512], f32, tag="gi")
            gf = lstm_sb.tile([B, 512], f32, tag="gf")
            gg = lstm_sb.tile([B, 512], f32, tag="gg")
            go = lstm_sb.tile([B, 512], f32, tag="go")
            t1 = lstm_sb.tile([B, 512], f32, tag="t1")
            t2 = lstm_sb.tile([B, 512], f32, tag="t2")
            tcl = lstm_sb.tile([B, 512], f32, tag="tc")
            h = lstm_sb.tile([B, 512], f32, tag="h", bufs=2)
            r = (t % 8) * B
            nc.scalar.activation(gf[:], gps[1][:], AF.Sigmoid)
            nc.scalar.activation(gi[:], gps[0][:], AF.Sigmoid)
            nc.scalar.activation(gg[:], gps[2][:], AF.Tanh)
            nc.scalar.activation(go[:], gps[3][:], AF.Sigmoid)
            del gps
            nc.vector.tensor_mul(t1[:], gf[:], c_sb[:])
            nc.vector.tensor_mul(t2[:], gi[:], gg[:])
            nc.vector.tensor_add(c_sb[:], t1[:], t2[:])
            nc.scalar.activation(tcl[:], c_sb[:], AF.Tanh)
            nc.vector.tensor_mul(h[:], go[:], tcl[:])
            return h

        def lstm_hT(t, h, hT_out):
            r = (t % 8) * B
            for k in range(4):
                tp = lstm_tp.tile([P, B], f32, tag="htp", name="htp")
                nc.tensor.transpose(tp[:], h[:, k * P:(k + 1) * P],
                                    ident[0:B, 0:B])
                nc.scalar.activation(hT_out[k][:, r:r + B], tp[:], AF.Identity)

        # ---- layer 1, with z2x chunks interleaved every 4 steps ----
        with tc.tile_pool(name="wpool", bufs=1) as wpool:
            wh1_sb = [wpool.tile([P, G4], f32r, tag=f"wh1_{k}", name=f"wh1_{k}")
                      for k in range(4)]
            wx2_sb = [wpool.tile([P, G4], f32r, tag=f"wx2_{k}", name=f"wx2_{k}")
                      for k in range(4)]
            for k in range(4):
                nc.sync.dma_start(wh1_sb[k][:], d["wh1"][k * P:(k + 1) * P, :])
                nc.sync.dma_start(wx2_sb[k][:], d["wx2b"][k * P:(k + 1) * P, :])
            bl2_sb = wpool.tile([1, G4], f32r, tag="bl2")
            nc.sync.dma_start(bl2_sb[:], d["wx2b"][U:U + 1, :])

            def z2x_unit_l1(q):
                # one gate-chunk of z2x token-chunk m (steps 4m..4m+3)
                m, g = q // 4, q % 4
                r0 = ((4 * m) % 8) * B
                ps = lstm_ps.tile([P, 512], f32, tag=f"gate{g}", name=f"z2ps{g}")
                gs = slice(g * 512, (g + 1) * 512)
                for k in range(4):
                    nc.tensor.matmul(ps[:], h1T[k][:, r0:r0 + P],
                                     wx2_sb[k][:, gs],
                                     start=(k == 0),
                                     stop=(k == 3 and _BIAS_ZERO))
                if not _BIAS_ZERO:
                    nc.tensor.matmul(ps[:], ones_r[:, 0:P], bl2_sb[:, gs],
                                     start=False, stop=True)
                ev = lstm_sb.tile([P, 512], f32r, tag="z2ev", bufs=2, name="z2ev")
                nc.vector.tensor_copy(ev[:], ps[:])
                nc.sync.dma_start(z2x_d[m * P:(m + 1) * P, gs], ev[:])

            h_prev_l1 = None
            for t in range(T):
                if t == 0:
                    prev = lambda k: h0T[:, k * B:(k + 1) * B]
                else:
                    prev = lambda k, _t=t: h1T[k][:, ((_t - 1) % 8) * B:
                                                  (((_t - 1) % 8) + 1) * B]
                h_t = lstm_step(t, z1x_d, wh1_sb, prev, h1T)
                if t >= 4:
                    z2x_unit_l1(t - 4)
                lstm_hT(t, h_t, h1T)
                h_prev_l1 = h_t

            for q in range(T - 4, T):
                z2x_unit_l1(q)

        # ---- layer 2, with FC chunks interleaved every 4 steps ----
        with tc.tile_pool(name="fcw", bufs=1) as fcw:
            wfc_sb = [fcw.tile([P, VSP], f32r, tag=f"wfc_{k}", name=f"wfc_{k}")
                      for k in range(4)]
            for k in range(4):
                nc.sync.dma_start(wfc_sb[k][:], d["wfcb"][k * P:(k + 1) * P, :])
            bfc_sb = fcw.tile([1, VSP], f32r, tag="bfc")
            nc.sync.dma_start(bfc_sb[:], d["wfcb"][U:U + 1, :])

            def fc_unit2(q):
                # one vocab-chunk of logits token-chunk m (steps 4m..4m+3)
                m, v = q // 8, q % 8
                r0 = ((4 * m) % 8) * B
                ps = ps_small.tile([P, 512], f32, tag="small", name="fps")
                vs = slice(v * 512, (v + 1) * 512)
                for k in range(4):
                    nc.tensor.matmul(ps[:], h2T[k][:, r0:r0 + P],
                                     wfc_sb[k][:, vs],
                                     start=(k == 0),
                                     stop=(k == 3 and _BIAS_ZERO))
                if not _BIAS_ZERO:
                    nc.tensor.matmul(ps[:], ones_r[:, 0:P], bfc_sb[:, vs],
                                     start=False, stop=True)
                ev = lstm_sb.tile([P, 512], f32, tag="fev", bufs=3, name="fev")
                nc.vector.tensor_copy(ev[:], ps[:])
                nc.sync.dma_start(d["logits"][m * P:(m + 1) * P, vs], ev[:])

            h_last = None
            for t in range(T):
                if t == 0:
                    prev = lambda k: h1T[k][:, 7 * B:8 * B]
                else:
                    prev = lambda k, _t=t: h2T[k][:, ((_t - 1) % 8) * B:
                                                  (((_t - 1) % 8) + 1) * B]
                h_last = lstm_step(t, z2x_d, wh2_sb, prev, h2T)
                if t >= 4:
                    fc_unit2(2 * (t - 4))
                    fc_unit2(2 * (t - 4) + 1)
                lstm_hT(t, h_last, h2T)

            for q in range(2 * (T - 4), 8 * (TOK // P)):
                fc_unit2(q)

            nc.sync.dma_start(d["hT"][:, :], h_last[:])
            nc.sync.dma_start(d["cT"][:, :], c_sb[:])


_NC_CACHE = {}
_BIAS_ZERO = True


def _get_program(bias_zero=True):
    key = ("nc", bias_zero)
    if key not in _NC_CACHE:
        _NC_CACHE[key] = build_program(bias_zero)
    return _NC_CACHE[key]


def _stage_inputs(x, salidas_encoder, estado_h, estado_c, emb, W1, b1, W2, b2,
                  Va, bV, Wx1, Wh1, bl1, Wx2, Wh2, bl2, Wfc, bfc):
    f = np.float32
    x = np.asarray(x)
    enc = np.ascontiguousarray(np.asarray(salidas_encoder, f).reshape(STOK, EU))
    h0 = np.ascontiguousarray(np.asarray(estado_h, f))
    c0 = np.ascontiguousarray(np.asarray(estado_c, f))
    emb = np.ascontiguousarray(np.asarray(emb, f))

    # decoder token ids, t-major: ids[t*32+b] = x[b, t]
    ids = np.ascontiguousarray(x.T.reshape(TOK, 1).astype(np.int32))

    # encoder tokens b-major with broadcast h0 rows appended (for u^T GEMM)
    encaugT = np.ascontiguousarray(
        np.concatenate([enc.T, np.repeat(h0, S, axis=0).T], axis=0))

    w12 = np.ascontiguousarray(np.concatenate([np.asarray(W1, f),
                                               np.asarray(W2, f)], axis=0))
    va = np.zeros((P, 8), f)
    va[:, 0::2] = np.asarray(Va, f).reshape(4, P).T
    va = np.ascontiguousarray(va)
    b12 = np.ascontiguousarray(
        (np.asarray(b1, f) + np.asarray(b2, f)).reshape(U, 1))

    wx1b = np.ascontiguousarray(np.concatenate(
        [np.asarray(Wx1, f), np.asarray(bl1, f).reshape(1, G4)], axis=0))
    wx2b = np.ascontiguousarray(np.concatenate(
        [np.asarray(Wx2, f), np.asarray(bl2, f).reshape(1, G4)], axis=0))
    wh1 = np.ascontiguousarray(np.asarray(Wh1, f))
    wh2 = np.ascontiguousarray(np.asarray(Wh2, f))

    common = dict(ids=ids, emb=emb, enc=enc, encaugT=encaugT, w12=w12, va=va,
                  b12=b12, h0=h0, c0=c0, wx1b=wx1b, wh1=wh1, wx2b=wx2b,
                  wh2=wh2)

    wfc = np.asarray(Wfc, f)
    bfc = np.asarray(bfc, f)
    in_maps = []
    for j in range(NC):
        wfcs = np.zeros((U + 1, VSP), f)
        wfcs[:U, :VS] = wfc[:, j * VS:(j + 1) * VS]
        wfcs[U, :VS] = bfc[j * VS:(j + 1) * VS]
        m = dict(common)
        m["wfcb"] = np.ascontiguousarray(wfcs)
        in_maps.append(m)
    return in_maps


def kernel(**inputs):
    bz = all(float(np.abs(np.asarray(inputs[k])).max()) == 0.0
             for k in ("bl1", "bl2", "bfc"))
    nc = _get_program(bias_zero=bz)
    in_maps = _stage_inputs(**inputs)
    res = run_bass_kernel_spmd(nc, in_maps, core_ids=list(range(NC)))
    outs = res.results

    parts = []
    for j in range(NC):
        lg = outs[j]["logits"][:, :VS]  # [2048 (t-major), 4000]
        parts.append(lg.reshape(T, B, VS).transpose(1, 0, 2))
    salidas = np.concatenate(parts, axis=2)

    o0 = outs[0]
    return (salidas, o0["hT"], o0["cT"], o0["pesos"].reshape(B, S, 1))


if __name__ == "__main__":
    print("building program...")
    _get_program()
    print("ok")
